# revision 1
# baseline (speedup 1.0000x reference)
"""Belief-propagation decoder kernel for TRN2 (8 NeuronCores, data-parallel batch).

v2: all irregular routing done with GPSIMD local_scatter (line-rate staged
scatter) + prefix-sum tricks; no ap_gather (which costs ~25ns/index).

Layout: 128 partitions = 8 check-groups (one per Q7 core) x 16 lanes
(batch b = lane % 8). Each group owns 64 checks (~395 edges). Per group the
edge stream is v-sorted; "runs" are maximal same-v segments, r = 0..R-1,
run r covers stream [a_r, a_{r+1}), variable v(r) strictly increasing.

Per iteration:
  L = cumsum(c2v)                      [DVE scan]
  LRUN[r] = L[a_r]                     [local_scatter]
  d[r] = LRUN[r+1]-LRUN[r]             [DVE] = per-group run sums
  S[v(r)] = d[r] (0 elsewhere)         [2x local_scatter halves]
  Wp = repl@llr + sel@S                [PE matmuls, PSUM] = llr + sum_g S_g
  W = copy(Wp)                         [DVE]
  WC[r+1] = W[v(r)]                    [local_scatter]
  Dl[r] = WC[r+1]-WC[r]                [DVE]
  Gd[a_r] = Dl[r] (0 elsewhere)        [local_scatter]
  v2c = cumsum(Gd) - c2v               [DVE custom scan-sub] = W(v(e)) - c2v
  t = tanh(0.5*v2c)                    [ACT]
  tc = sgn*clip(|t|)                   [DVE custom]
  tp[slot(e)] = tc[e]                  [local_scatter x NP/512]
  tpf = tp + padconst                  [DVE]  (padconst: K_c / 1.0 at pads)
  P = product-tree over D slots        [DVE x log2(D)]
  a2 = (tpf+P)^2, b2 = (tpf-P)^2       [DVE custom, P broadcast]
  La = Ln(a2), Lb = Ln(b2)             [ACT]
  c2v_p = clip(0.5*(La-Lb))            [DVE custom] = 2*atanh(clip(P/t))
  c2v[e] = c2v_p[slot(e)]              [local_scatter]
"""
import sys, os
sys.path.insert(0, "/opt/trn_rl_repo")
import numpy as np

import concourse.bass as bass
import concourse.bacc as bacc
import concourse.tile as tile
from concourse import mybir
from concourse import bass_utils

# ----------------------------------------------------------------- constants
N_VAR, N_CHK, N_INFO, N_ITER, BATCH = 1024, 512, 512, 5, 64
EPS = 1e-7
CLIP = 1.0 - 1e-6
C2V_BOUND = float(2.0 * np.arctanh(np.float32(CLIP)))
N_CORES = 8
N_GROUPS = 8
CHK_PER_G = 66          # check SLOTS per group; big checks (deg>15) use 2
D_PAD = 16
B_LOC = 8

# ------------------------------------------------------- custom DVE ops
from concourse.dve_spec import (
    Spec, Src0, Src1, C0, C1, C2, Zero, One, scan, AluOp,
    select, maxx, minn, sq, lower, _has_src1 as has_src1,
)
import concourse.dve_ops as dve_ops
from concourse.dve_ops import DveOp, OPS
from concourse.dve_uop import DveOpSpec


def _register(name, spec, subdim=False):
    if name in dve_ops._SUB_OPCODE_FOR_NAME:
        for op in OPS:
            if op.name == name:
                return op
        raise RuntimeError(name)
    shas = {}
    for ver in ("v3", "v4"):
        try:
            tmp = DveOpSpec(name=name, uops=lower(spec, ver=ver),
                            rd1_en=has_src1(spec))
            shas[ver] = tmp.sha(ver)
        except Exception:
            pass
    op = DveOp(name, spec, subdim=subdim, uops_sha=shas)
    OPS.append(op)
    dve_ops.CUSTOM_DVE_SPECS[name] = spec
    dve_ops._SUB_OPCODE_FOR_NAME[name] = dve_ops._CUSTOM_DVE_ROW_BASE + len(OPS) - 1
    assert dve_ops._SUB_OPCODE_FOR_NAME[name] < 0x20
    return op


ANT_CUMSUM = _register("ANT_BP_CUMSUM", Spec(
    body=scan(AluOp.ADD, Src0),
    reference=lambda in0, in1, s0, s1, imm2: np.cumsum(in0, axis=-1),
))
# out = cumsum(in0) - in1
ANT_SCANSUB = _register("ANT_BP_SCANSUB", Spec(
    body=scan(AluOp.ADD, Src0) - Src1,
    reference=lambda in0, in1, s0, s1, imm2: np.cumsum(in0, axis=-1) - in1,
))
# sgn(x)*clip(|x|, s0, s1), sgn(-0.0)=+1 (matches jnp.where(t >= 0, 1, -1))
ANT_SGNCLIP = _register("ANT_BP_SGNCLIP", Spec(
    body=select(Src0 < Zero, Zero - One, One)
         * minn(maxx(maxx(Src0, Zero - Src0), C0), C1),
    reference=lambda in0, in1, s0, s1, imm2:
        np.where(in0 < 0, -1.0, 1.0).astype(np.float32)
        * np.clip(np.abs(in0), s0, s1),
))
ANT_SQADD = _register("ANT_BP_SQADD", Spec(
    body=sq(Src0 + Src1),
    reference=lambda in0, in1, s0, s1, imm2: (in0 + in1.reshape(in0.shape)) ** 2,
))
ANT_SQSUB = _register("ANT_BP_SQSUB", Spec(
    body=sq(Src0 - Src1),
    reference=lambda in0, in1, s0, s1, imm2: (in0 - in1.reshape(in0.shape)) ** 2,
))
# clip((in0-in1)*imm2, s0, s1)
ANT_SUBCLIPSCALE = _register("ANT_BP_SUBCLIPSCALE", Spec(
    body=minn(maxx((Src0 - Src1) * C2, C0), C1),
    reference=lambda in0, in1, s0, s1, imm2: np.clip((in0 - in1) * imm2, s0, s1),
))


# ------------------------------------------------------------ host-side prep
def prep(H: np.ndarray) -> dict:
    H = np.asarray(H)
    assert H.shape == (N_CHK, N_VAR)
    deg = (H > 0).sum(1).astype(int)
    assert deg.max() <= 30
    NP = CHK_PER_G * D_PAD
    slots_of = {c: (2 if deg[c] > 15 else 1) for c in range(N_CHK)}
    order = np.argsort(-deg, kind="stable")
    ge = [0] * N_GROUPS
    gcnt = [0] * N_GROUPS
    grp = np.zeros(N_CHK, int)
    for c in order:
        g = min(range(N_GROUPS),
                key=lambda g: ge[g]
                if gcnt[g] + slots_of[c] <= CHK_PER_G else 1 << 30)
        grp[c] = g
        ge[g] += deg[c]
        gcnt[g] += slots_of[c]
    ES = -(-max(max(ge), 1) // 2) * 2          # stream length (even)

    # per-group metadata
    g_edges = []    # (checks, sorted (v, c) list)
    g_runs = []     # ([(a_r, v_r)...], Eg)
    for g in range(N_GROUPS):
        checks = np.where(grp == g)[0]
        es = []
        for c in checks:
            for v in np.where(H[c] > 0)[0]:
                es.append((int(v), int(c)))
        es.sort()
        g_edges.append((checks, es))
        runs = []
        for e, (v, c) in enumerate(es):
            if not runs or runs[-1][1] != v:
                runs.append((e, v))
        g_runs.append((runs, len(es)))
    R_max = max(len(r) for r, _ in g_runs)
    RP = -(-R_max // 2) * 2                     # run slots, even

    def pairs(n):
        return np.full((128, 2 * n), -1, np.int16)

    lrun_idx = pairs(ES + 2)        # source L[0..ES+1] -> LRUN[r] at a_r
    sA_idx = pairs(RP)              # source d[r] -> S[v(r)] (v < 512)
    sB_idx = pairs(RP)
    wc_idx = pairs(N_VAR)           # source W[v] -> WC[r+1]
    gd_idx = pairs(RP)              # source Dl[r] -> Gd[a_r]
    NPH = NP // 2                   # tp chunk width (fp32), dst fits scratch
    tp_idx = [pairs(ES), pairs(ES)]  # source tc[e] -> tp chunk
    bk_idx = pairs(NP)              # source c2v_p[slot] -> c2v[e]
    padc = np.zeros((128, NP), np.float32)
    partner = np.full((128, CHK_PER_G), -1, np.int64)
    mask1 = np.ones((128, CHK_PER_G), np.float32)

    for g in range(N_GROUPS):
        checks, es = g_edges[g]
        runs, Eg = g_runs[g]
        rows = slice(16 * g, 16 * g + 16)

        def put(arr, src_fp, dst_fp):
            arr[rows, 2 * src_fp] = 2 * dst_fp
            arr[rows, 2 * src_fp + 1] = 2 * dst_fp + 1

        for r, (a_r, v_r) in enumerate(runs):
            put(lrun_idx, a_r, r)
            if v_r < 512:
                put(sA_idx, r, v_r)
            else:
                put(sB_idx, r, v_r - 512)
            put(wc_idx, v_r, r + 1)
            put(gd_idx, r, a_r)
        put(lrun_idx, Eg, len(runs))            # closing boundary

        # assign slot positions: big checks take slot pairs (i, i+1)
        cpos = {}
        nxt_slot = 0
        for c in checks:
            cpos[c] = nxt_slot
            if slots_of[c] == 2:
                partner[rows, nxt_slot] = nxt_slot + 1
                partner[rows, nxt_slot + 1] = nxt_slot
                mask1[rows, nxt_slot] = 0.0
                mask1[rows, nxt_slot + 1] = 0.0
            nxt_slot += slots_of[c]
        assert nxt_slot <= CHK_PER_G
        dslot = {c: 0 for c in checks}
        for e, (v, c) in enumerate(es):
            s = cpos[c] * D_PAD + dslot[c]
            dslot[c] += 1
            put(tp_idx[s // NPH], e, s % NPH)
            put(bk_idx, s, e)
        for c in checks:
            dd = dslot[c]            # = deg(c), spans 1 or 2 slots
            base = cpos[c] * D_PAD
            k = np.float32(np.float64(CLIP) ** (N_VAR - dd))
            padc[rows, base + dd] = k
            for j in range(dd + 1, slots_of[c] * D_PAD):
                padc[rows, base + j] = 1.0

    # selector matrices: partitions p = 16g + j, batch lane b = j % 8,
    # source lanes j < 8
    sel = np.zeros((128, 128), np.float32)
    for k in range(128):
        if k % 16 < 8:
            for m in range(128):
                if m % 16 % 8 == k % 16:
                    sel[k, m] = 1.0
    repl = np.zeros((8, 128), np.float32)
    for b in range(8):
        for m in range(128):
            if m % 16 % 8 == b:
                repl[b, m] = 1.0

    has_split = bool((mask1 == 0.0).any())
    pp_idx = np.full((128, 2 * CHK_PER_G), -1, np.int16)
    for p in range(128):
        for sl in range(CHK_PER_G):
            if partner[p, sl] >= 0:
                pp_idx[p, 2 * sl] = 2 * partner[p, sl]
                pp_idx[p, 2 * sl + 1] = 2 * partner[p, sl] + 1
    return dict(ES=ES, RP=RP, NP=NP, D_PAD=D_PAD, has_split=has_split,
                lrun_idx=lrun_idx, sA_idx=sA_idx, sB_idx=sB_idx,
                wc_idx=wc_idx, gd_idx=gd_idx, tp_idx=tp_idx, bk_idx=bk_idx,
                pp_idx=pp_idx, mask1=mask1,
                padc=padc, sel=sel, repl=repl,
                _dbg=dict(g_edges=g_edges, g_runs=g_runs))


IDX_NAMES = ["lrun_idx", "sA_idx", "sB_idx", "wc_idx", "gd_idx", "bk_idx"]


# ------------------------------------------------------------- device program
def build_nc(pre: dict, n_iters: int = N_ITER, repeat: int = 1):
    ES, RP, NP = pre["ES"], pre["RP"], pre["NP"]
    NPH = NP // 2
    NCH = 2
    f32 = mybir.dt.float32
    i16 = mybir.dt.int16

    nc = bacc.Bacc("TRN2", target_bir_lowering=False, debug=False)
    x_d = nc.dram_tensor("x", [B_LOC, N_VAR], f32, kind="ExternalInput")
    sig_d = nc.dram_tensor("sigma2", [1, 1], f32, kind="ExternalInput")
    idx_d = {k: nc.dram_tensor(k, list(pre[k].shape), i16, kind="ExternalInput")
             for k in IDX_NAMES}
    tp_d = [nc.dram_tensor(f"tp_idx{j}", list(pre["tp_idx"][j].shape), i16,
                           kind="ExternalInput") for j in range(NCH)]
    padc_d = nc.dram_tensor("padc", [128, NP], f32, kind="ExternalInput")
    pp_d = nc.dram_tensor("pp_idx", [128, 2 * CHK_PER_G], i16, kind="ExternalInput")
    m1_d = nc.dram_tensor("mask1", [128, CHK_PER_G], f32, kind="ExternalInput")
    sel_d = nc.dram_tensor("sel", [128, 128], f32, kind="ExternalInput")
    repl_d = nc.dram_tensor("repl", [8, 128], f32, kind="ExternalInput")
    out_d = nc.dram_tensor("out", [B_LOC, N_INFO], f32, kind="ExternalOutput")

    def i16v(ap):
        return ap.bitcast(i16)

    with tile.TileContext(nc) as tc:
        with tc.tile_pool(name="main", bufs=1) as pool, \
             tc.tile_pool(name="ps", bufs=1, space="PSUM") as psp:
            x_sb = pool.tile([B_LOC, N_VAR], f32)
            sig_sb = pool.tile([B_LOC, 1], f32)
            scale = pool.tile([B_LOC, 1], f32)
            llr = pool.tile([B_LOC, N_VAR], f32)
            idx_sb = {k: pool.tile(list(pre[k].shape), i16, name=k + "_sb")
                      for k in IDX_NAMES}
            tp_sb = [pool.tile(list(pre["tp_idx"][j].shape), i16, name=f"tpi{j}_sb")
                     for j in range(NCH)]
            padc_sb = pool.tile([128, NP], f32)
            pp_sb = pool.tile([128, 2 * CHK_PER_G], i16)
            m1_sb = pool.tile([128, CHK_PER_G], f32)
            Pp = pool.tile([128, CHK_PER_G], f32)
            Pfin = pool.tile([128, CHK_PER_G], f32)
            sel = pool.tile([128, 128], f32)
            repl = pool.tile([8, 128], f32)

            c2v = pool.tile([128, ES], f32)
            L = pool.tile([128, ES + 2], f32)
            LRUN = pool.tile([128, RP + 2], f32)
            d_t = pool.tile([128, RP], f32)
            S = pool.tile([128, N_VAR], f32)
            W = pool.tile([128, N_VAR], f32)
            WC = pool.tile([128, RP + 2], f32)
            Dl = pool.tile([128, RP], f32)
            Gd = pool.tile([128, ES], f32)
            v2c = pool.tile([128, ES], f32)
            t = pool.tile([128, ES], f32)
            tcl = pool.tile([128, ES], f32)
            tp = pool.tile([128, NP], f32)
            tpf = pool.tile([128, NP], f32)
            tree = []
            w = NP // 2
            while w >= CHK_PER_G:
                tree.append(pool.tile([128, w], f32, name=f"tree{w}"))
                w //= 2
            ab2 = pool.tile([128, 2 * NP], f32)
            lab = pool.tile([128, 2 * NP], f32)
            c2vp = pool.tile([128, NP], f32)
            out_sb = pool.tile([B_LOC, N_INFO], f32)
            Wp = psp.tile([128, N_VAR], f32)

            # ---- loads
            nc.sync.dma_start(out=x_sb[:], in_=x_d.ap())
            sig_b = bass.AP(tensor=sig_d.ap().tensor, offset=0,
                            ap=[[0, B_LOC], [1, 1]])
            nc.sync.dma_start(out=sig_sb[:], in_=sig_b)
            for k in IDX_NAMES:
                nc.sync.dma_start(out=idx_sb[k][:], in_=idx_d[k].ap())
            for j in range(NCH):
                nc.sync.dma_start(out=tp_sb[j][:], in_=tp_d[j].ap())
            nc.sync.dma_start(out=padc_sb[:], in_=padc_d.ap())
            nc.sync.dma_start(out=pp_sb[:], in_=pp_d.ap())
            nc.sync.dma_start(out=m1_sb[:], in_=m1_d.ap())
            nc.sync.dma_start(out=sel[:], in_=sel_d.ap())
            nc.sync.dma_start(out=repl[:], in_=repl_d.ap())

            # llr = x * (-4 / sigma2)
            nc.vector.reciprocal(out=scale[:], in_=sig_sb[:])
            nc.vector.tensor_scalar_mul(out=scale[:], in0=scale[:], scalar1=-4.0)
            nc.vector.tensor_scalar_mul(out=llr[:], in0=x_sb[:], scalar1=scale[:])

            def lscat(dst_fp, src_fp, idx, n_dst_fp, n_src_fp):
                nc.gpsimd.local_scatter(
                    i16v(dst_fp), i16v(src_fp), idx,
                    channels=128, num_elems=2 * n_dst_fp, num_idxs=2 * n_src_fp)

            for _rep in range(repeat):
                nc.vector.memset(c2v[:], 0.0)
                nc.vector.memset(L[:], 0.0)

                for it in range(n_iters + 1):
                    if it > 0:
                        nc.vector._custom_dve(ANT_CUMSUM, out=L[:, 1:ES + 1],
                                              in0=c2v[:])
                        lscat(LRUN[:], L[:], idx_sb["lrun_idx"][:],
                              RP + 2, ES + 2)
                        nc.vector.tensor_tensor(out=d_t[:], in0=LRUN[:, 1:RP + 1],
                                                in1=LRUN[:, 0:RP],
                                                op=mybir.AluOpType.subtract)
                        lscat(S[:, 0:512], d_t[:], idx_sb["sA_idx"][:], 512, RP)
                        lscat(S[:, 512:1024], d_t[:], idx_sb["sB_idx"][:], 512, RP)
                    for h in range(2):
                        cols = slice(512 * h, 512 * (h + 1))
                        nc.tensor.matmul(Wp[:, cols], repl[:], llr[:, cols],
                                         start=True, stop=(it == 0))
                        if it > 0:
                            nc.tensor.matmul(Wp[:, cols], sel[:], S[:, cols],
                                             start=False, stop=True)
                    if it == n_iters:
                        break
                    nc.vector.tensor_copy(out=W[:], in_=Wp[:])
                    lscat(WC[:], W[:], idx_sb["wc_idx"][:], RP + 2, N_VAR)
                    nc.vector.tensor_tensor(out=Dl[:], in0=WC[:, 1:RP + 1],
                                            in1=WC[:, 0:RP],
                                            op=mybir.AluOpType.subtract)
                    lscat(Gd[:], Dl[:], idx_sb["gd_idx"][:], ES, RP)
                    nc.vector._custom_dve(ANT_SCANSUB, out=v2c[:], in0=Gd[:],
                                          in1=c2v[:])
                    nc.scalar.activation(out=t[:], in_=v2c[:],
                                         func=mybir.ActivationFunctionType.Tanh,
                                         scale=0.5)
                    nc.vector._custom_dve(ANT_SGNCLIP, out=tcl[:], in0=t[:],
                                          s0=EPS, s1=CLIP)
                    for j in range(NCH):
                        lscat(tp[:, NPH * j:NPH * (j + 1)], tcl[:],
                              tp_sb[j][:], NPH, ES)
                    nc.vector.tensor_tensor(out=tpf[:], in0=tp[:], in1=padc_sb[:],
                                            op=mybir.AluOpType.add)
                    src, dd = tpf, D_PAD
                    for rbuf in tree:
                        s3 = src[:].rearrange("p (c d) -> p c d", d=dd)
                        nc.vector.tensor_tensor(out=rbuf[:], in0=s3[:, :, 0::2],
                                                in1=s3[:, :, 1::2],
                                                op=mybir.AluOpType.mult)
                        src, dd = rbuf, dd // 2
                    P = src
                    if pre["has_split"]:
                        lscat(Pp[:], P[:], pp_sb[:], CHK_PER_G, CHK_PER_G)
                        nc.vector.tensor_tensor(out=Pp[:], in0=Pp[:], in1=m1_sb[:],
                                                op=mybir.AluOpType.add)
                        nc.vector.tensor_tensor(out=Pfin[:], in0=P[:], in1=Pp[:],
                                                op=mybir.AluOpType.mult)
                        P = Pfin
                    Pb = bass.AP(tensor=P.tensor, offset=P.offset,
                                 ap=[P.ap[0], [1, CHK_PER_G], [0, D_PAD]])
                    nc.vector._custom_dve(ANT_SQADD, out=ab2[:, 0:NP],
                                          in0=tpf[:], in1=Pb)
                    nc.vector._custom_dve(ANT_SQSUB, out=ab2[:, NP:2 * NP],
                                          in0=tpf[:], in1=Pb)
                    nc.scalar.activation(out=lab[:], in_=ab2[:],
                                         func=mybir.ActivationFunctionType.Ln)
                    nc.vector._custom_dve(ANT_SUBCLIPSCALE, out=c2vp[:],
                                          in0=lab[:, 0:NP], in1=lab[:, NP:2 * NP],
                                          s0=-C2V_BOUND, s1=C2V_BOUND, imm2=0.5)
                    lscat(c2v[:], c2vp[:], idx_sb["bk_idx"][:], ES, NP)

                nc.scalar.activation(out=out_sb[:], in_=Wp[0:B_LOC, 0:N_INFO],
                                     func=mybir.ActivationFunctionType.Sigmoid,
                                     scale=-1.0)
            nc.sync.dma_start(out=out_d.ap(), in_=out_sb[:])
    nc.compile()
    return nc


def make_in_maps(pre, x, sigma2):
    sigma2 = np.asarray(sigma2, np.float32).reshape(1, 1)
    in_maps = []
    for c in range(N_CORES):
        m = {"x": np.asarray(x[c * B_LOC:(c + 1) * B_LOC], np.float32),
             "sigma2": sigma2, "padc": pre["padc"],
             "sel": pre["sel"], "repl": pre["repl"]}
        m["pp_idx"] = pre["pp_idx"]
        m["mask1"] = pre["mask1"]
        for k in IDX_NAMES:
            m[k] = pre[k]
        for j in range(len(pre["tp_idx"])):
            m[f"tp_idx{j}"] = pre["tp_idx"][j]
        in_maps.append(m)
    return in_maps


# ------------------------------------------------------------------ reference
def ref_numpy(inputs, H, sigma2, n_iter=N_ITER):
    llr = -4.0 * inputs / np.asarray(sigma2).reshape(-1)
    B = inputs.shape[0]
    mask = H[None]
    c2v = np.zeros((B, N_CHK, N_VAR), np.float32)
    for _ in range(n_iter):
        v2c = (llr[:, None, :] + c2v.sum(1, keepdims=True) - c2v) * mask
        tt = np.tanh(0.5 * v2c)
        tt = np.where(mask > 0, tt, 1.0)
        sgn = np.where(tt >= 0, 1.0, -1.0)
        tt = sgn * np.clip(np.abs(tt), EPS, CLIP)
        ext = np.prod(tt, axis=2, keepdims=True) / tt
        c2v = 2.0 * np.arctanh(np.clip(ext, -CLIP, CLIP)) * mask
    total = llr + c2v.sum(1)
    return (1.0 / (1.0 + np.exp(total[:, :N_INFO]))).astype(np.float32)


# ------------------------------------------------------------------- pipeline
def run(inputs, H, sigma2, n_iters=N_ITER, repeat=1, nc=None, pre=None):
    inputs = np.asarray(inputs, np.float32)
    H = np.asarray(H, np.float32)
    if pre is None:
        pre = prep(H)
    if nc is None:
        nc = build_nc(pre, n_iters=n_iters, repeat=repeat)
    in_maps = make_in_maps(pre, inputs, sigma2)
    res = bass_utils.run_bass_kernel_spmd(nc, in_maps, core_ids=list(range(N_CORES)))
    out = np.concatenate([res.results[c]["out"] for c in range(N_CORES)], axis=0)
    return out


# ------------------------------------------------------------------ entry
_CACHE = {}


def kernel(inputs, H, sigma2):
    """Full-input entry point: inputs [64,1024] f32, H [512,1024] f32,
    sigma2 [1] f32 -> [64, 512] f32 (matches reference.reference)."""
    inputs = np.ascontiguousarray(np.asarray(inputs, np.float32))
    H = np.asarray(H, np.float32)
    pre = prep(H)
    key = (pre["ES"], pre["RP"], pre["NP"], pre["has_split"])
    nc = _CACHE.get(key)
    if nc is None:
        nc = build_nc(pre)
        _CACHE[key] = nc
    out = run(inputs, H, sigma2, nc=nc, pre=pre)
    return np.ascontiguousarray(out.astype(np.float32))



# revision 28
# speedup vs baseline: 99.8414x; 99.8414x over previous
"""Belief-propagation decoder kernel for TRN2 (8 NeuronCores, data-parallel batch).

v3: 16 check-groups x 8 batch lanes (v2 duplicated each batch twice across
16 lanes/group — pure waste on GPSIMD/DVE). All stream widths (ES/RP/NP)
halve. repl@llr hoisted out of the loop (W0). Activation table loads hidden
behind dummy activations on the idle ACT engine. Index tables packed into a
single DMA issued on a second queue.

Layout: 128 partitions = 16 check-groups x 8 lanes (batch b = lane). Each
group owns ~32 checks (~192 edges). Per group the edge stream is v-sorted;
"runs" are maximal same-v segments, r = 0..R-1, run r covers stream
[a_r, a_{r+1}), variable v(r) strictly increasing.

Per iteration:
  L = cumsum(c2v)                      [DVE scan]
  LRUN[r] = L[a_r]                     [local_scatter]
  d[r] = LRUN[r+1]-LRUN[r]             [DVE] = per-group run sums
  S[v(r)] = d[r] (0 elsewhere)         [2x local_scatter halves]
  Wp = sel@S                           [PE matmul, PSUM] = sum_g S_g
  W = W0 + Wp                          [DVE]  (W0 = repl@llr, hoisted)
  WC[r+1] = W[v(r)]                    [local_scatter]
  Dl[r] = WC[r+1]-WC[r]                [DVE]
  Gd[a_r] = Dl[r] (0 elsewhere)        [local_scatter]
  v2c = cumsum(Gd) - c2v               [DVE custom scan-sub] = W(v(e)) - c2v
  t = tanh(0.5*v2c)                    [ACT]
  tc = sgn*clip(|t|)                   [DVE custom]
  tp[slot(e)] = tc[e]                  [local_scatter]
  tpf = tp + padconst                  [DVE]  (padconst: K_c / 1.0 at pads)
  P = product-tree over D slots        [DVE x log2(D)]
  a2 = (tpf+P)^2, b2 = (tpf-P)^2       [DVE custom, P broadcast]
  La = Ln(a2), Lb = Ln(b2)             [ACT]
  c2v_p = clip(0.5*(La-Lb))            [DVE custom] = 2*atanh(clip(P/t))
  c2v[e] = c2v_p[slot(e)]              [local_scatter]
"""
import sys, os
sys.path.insert(0, "/opt/trn_rl_repo")
import numpy as np

import concourse.bass as bass
import concourse.bacc as bacc
import concourse.tile as tile
from concourse import mybir
from concourse import bass_utils

# ----------------------------------------------------------------- constants
N_VAR, N_CHK, N_INFO, N_ITER, BATCH = 1024, 512, 512, 5, 64
EPS = 1e-7
CLIP = 1.0 - 1e-6
C2V_BOUND = float(2.0 * np.arctanh(np.float32(CLIP)))
N_CORES = 8
N_GROUPS = 16
CHK_PER_G = 34          # check SLOTS per group; big checks (deg>15) use 2
D_PAD = 16
B_LOC = 8

# ------------------------------------------------------- custom DVE ops
from concourse.dve_spec import (
    Spec, Src0, Src1, C0, C1, C2, Zero, One, scan, AluOp,
    select, maxx, minn, sq, lower, _has_src1 as has_src1,
)
import concourse.dve_ops as dve_ops
from concourse.dve_ops import DveOp, OPS
from concourse.dve_uop import DveOpSpec


def _register(name, spec, subdim=False):
    if name in dve_ops._SUB_OPCODE_FOR_NAME:
        for op in OPS:
            if op.name == name:
                return op
        raise RuntimeError(name)
    shas = {}
    for ver in ("v3", "v4"):
        try:
            tmp = DveOpSpec(name=name, uops=lower(spec, ver=ver),
                            rd1_en=has_src1(spec))
            shas[ver] = tmp.sha(ver)
        except Exception:
            pass
    op = DveOp(name, spec, subdim=subdim, uops_sha=shas)
    OPS.append(op)
    dve_ops.CUSTOM_DVE_SPECS[name] = spec
    dve_ops._SUB_OPCODE_FOR_NAME[name] = dve_ops._CUSTOM_DVE_ROW_BASE + len(OPS) - 1
    assert dve_ops._SUB_OPCODE_FOR_NAME[name] < 0x20
    return op


ANT_CUMSUM = _register("ANT_BP_CUMSUM", Spec(
    body=scan(AluOp.ADD, Src0),
    reference=lambda in0, in1, s0, s1, imm2: np.cumsum(in0, axis=-1),
))
# out = cumsum(in0) - in1
ANT_SCANSUB = _register("ANT_BP_SCANSUB", Spec(
    body=scan(AluOp.ADD, Src0) - Src1,
    reference=lambda in0, in1, s0, s1, imm2: np.cumsum(in0, axis=-1) - in1,
))
# sgn(x)*clip(|x|, s0, s1), sgn(-0.0)=+1 (matches jnp.where(t >= 0, 1, -1))
ANT_SGNCLIP = _register("ANT_BP_SGNCLIP", Spec(
    body=select(Src0 < Zero, Zero - One, One)
         * minn(maxx(maxx(Src0, Zero - Src0), C0), C1),
    reference=lambda in0, in1, s0, s1, imm2:
        np.where(in0 < 0, -1.0, 1.0).astype(np.float32)
        * np.clip(np.abs(in0), s0, s1),
))
ANT_SQADD = _register("ANT_BP_SQADD", Spec(
    body=sq(Src0 + Src1),
    reference=lambda in0, in1, s0, s1, imm2: (in0 + in1.reshape(in0.shape)) ** 2,
))
ANT_SQSUB = _register("ANT_BP_SQSUB", Spec(
    body=sq(Src0 - Src1),
    reference=lambda in0, in1, s0, s1, imm2: (in0 - in1.reshape(in0.shape)) ** 2,
))
# clip((in0-in1)*imm2, s0, s1)
ANT_SUBCLIPSCALE = _register("ANT_BP_SUBCLIPSCALE", Spec(
    body=minn(maxx((Src0 - Src1) * C2, C0), C1),
    reference=lambda in0, in1, s0, s1, imm2: np.clip((in0 - in1) * imm2, s0, s1),
))


# ------------------------------------------------------------ host-side prep
def prep(H: np.ndarray) -> dict:
    H = np.asarray(H)
    assert H.shape == (N_CHK, N_VAR)
    deg = (H > 0).sum(1).astype(int)
    assert deg.max() <= 30
    NP = CHK_PER_G * D_PAD
    slots_of = {c: (2 if deg[c] > 15 else 1) for c in range(N_CHK)}
    order = np.argsort(-deg, kind="stable")
    ge = [0] * N_GROUPS
    gcnt = [0] * N_GROUPS
    grp = np.zeros(N_CHK, int)
    for c in order:
        g = min(range(N_GROUPS),
                key=lambda g: ge[g]
                if gcnt[g] + slots_of[c] <= CHK_PER_G else 1 << 30)
        grp[c] = g
        ge[g] += deg[c]
        gcnt[g] += slots_of[c]
    ES = -(-max(max(ge), 1) // 2) * 2          # stream length (even)

    # per-group metadata
    g_edges = []    # (checks, sorted (v, c) list)
    g_runs = []     # ([(a_r, v_r)...], Eg)
    for g in range(N_GROUPS):
        checks = np.where(grp == g)[0]
        es = []
        for c in checks:
            for v in np.where(H[c] > 0)[0]:
                es.append((int(v), int(c)))
        es.sort()
        g_edges.append((checks, es))
        runs = []
        for e, (v, c) in enumerate(es):
            if not runs or runs[-1][1] != v:
                runs.append((e, v))
        g_runs.append((runs, len(es)))
    R_max = max(len(r) for r, _ in g_runs)
    RP = -(-R_max // 2) * 2                     # run slots, even

    def pairs(n):
        return np.full((128, 2 * n), -1, np.int16)

    lrun_idx = pairs(ES + 2)        # source L[0..ES+1] -> LRUN[r] at a_r
    sSA_idx = np.full((128, RP), -1, np.int16)  # bf16: d[r] -> S[v(r)], v<512
    sSB_idx = np.full((128, RP), -1, np.int16)  # bf16: d[r] -> S[v(r)-512]
    wcA_idx = pairs(512)            # source W[0:512][v] -> WCa[r+1]  (v < 512)
    wcB_idx = pairs(512)            # source W[512:][v-512] -> WCb[r+1]
    gd_idx = pairs(RP)              # source Dl[r] -> Gd[a_r]
    tp_idx = pairs(ES)              # source tc[e] -> tp slot
    bk_idx = pairs(NP)              # source c2v_p[slot] -> c2v[e]
    padc = np.zeros((128, NP), np.float32)
    partner = np.full((128, CHK_PER_G), -1, np.int64)
    mask1 = np.ones((128, CHK_PER_G), np.float32)

    for g in range(N_GROUPS):
        checks, es = g_edges[g]
        runs, Eg = g_runs[g]
        rows = slice(B_LOC * g, B_LOC * g + B_LOC)

        def put(arr, src_fp, dst_fp):
            arr[rows, 2 * src_fp] = 2 * dst_fp
            arr[rows, 2 * src_fp + 1] = 2 * dst_fp + 1

        for r, (a_r, v_r) in enumerate(runs):
            put(lrun_idx, a_r, r)
            if v_r < 512:
                sSA_idx[rows, r] = v_r
            else:
                sSB_idx[rows, r] = v_r - 512
            if v_r < 512:
                put(wcA_idx, v_r, r + 1)
            else:
                put(wcB_idx, v_r - 512, r + 1)
            put(gd_idx, r, a_r)
        put(lrun_idx, Eg, len(runs))            # closing boundary

        # assign slot positions: big checks take slot pairs (i, i+1)
        cpos = {}
        nxt_slot = 0
        for c in checks:
            cpos[c] = nxt_slot
            if slots_of[c] == 2:
                partner[rows, nxt_slot] = nxt_slot + 1
                partner[rows, nxt_slot + 1] = nxt_slot
                mask1[rows, nxt_slot] = 0.0
                mask1[rows, nxt_slot + 1] = 0.0
            nxt_slot += slots_of[c]
        assert nxt_slot <= CHK_PER_G
        dslot = {c: 0 for c in checks}
        for e, (v, c) in enumerate(es):
            s = cpos[c] * D_PAD + dslot[c]
            dslot[c] += 1
            put(tp_idx, e, s)
            put(bk_idx, s, e)
        for c in checks:
            dd = dslot[c]            # = deg(c), spans 1 or 2 slots
            base = cpos[c] * D_PAD
            k = np.float32(np.float64(CLIP) ** (N_VAR - dd))
            padc[rows, base + dd] = k
            for j in range(dd + 1, slots_of[c] * D_PAD):
                padc[rows, base + j] = 1.0

    # selector matrices: partition p = 8g + b, batch lane b = p % 8
    sel = np.zeros((128, 128), np.float32)
    for k in range(128):
        for m in range(128):
            if m % B_LOC == k % B_LOC:
                sel[k, m] = 1.0
    repl = np.zeros((8, 128), np.float32)
    for b in range(8):
        for m in range(128):
            if m % B_LOC == b:
                repl[b, m] = 1.0

    has_split = bool((mask1 == 0.0).any())
    pp_idx = np.full((128, 2 * CHK_PER_G), -1, np.int16)
    for p in range(128):
        for sl in range(CHK_PER_G):
            if partner[p, sl] >= 0:
                pp_idx[p, 2 * sl] = 2 * partner[p, sl]
                pp_idx[p, 2 * sl + 1] = 2 * partner[p, sl] + 1

    # ---- packed tables: one i16 blob + one f32 blob (single DMAs)
    i16_parts = [("lrun_idx", lrun_idx), ("sSA_idx", sSA_idx),
                 ("sSB_idx", sSB_idx),
                 ("wcA_idx", wcA_idx), ("wcB_idx", wcB_idx),
                 ("gd_idx", gd_idx), ("tp_idx", tp_idx),
                 ("bk_idx", bk_idx), ("pp_idx", pp_idx)]
    i16_off = {}
    off = 0
    for nm, arr in i16_parts:
        i16_off[nm] = (off, arr.shape[1])
        off += arr.shape[1]
    i16_pack = np.concatenate([a for _, a in i16_parts], axis=1)
    f32_parts = [("padc", padc), ("mask1", mask1), ("sel", sel)]
    f32_off = {}
    off = 0
    for nm, arr in f32_parts:
        f32_off[nm] = (off, arr.shape[1])
        off += arr.shape[1]
    f32_pack = np.concatenate([a for _, a in f32_parts], axis=1)

    return dict(ES=ES, RP=RP, NP=NP, D_PAD=D_PAD, has_split=has_split,
                i16_pack=i16_pack, i16_off=i16_off,
                f32_pack=f32_pack, f32_off=f32_off,
                repl=repl,
                _dbg=dict(g_edges=g_edges, g_runs=g_runs))


# ------------------------------------------------------------- device program
def build_nc(pre: dict, n_iters: int = N_ITER, repeat: int = 1):
    ES, RP, NP = pre["ES"], pre["RP"], pre["NP"]
    f32 = mybir.dt.float32
    i16 = mybir.dt.int16
    AFT = mybir.ActivationFunctionType

    nc = bacc.Bacc("TRN2", target_bir_lowering=False, debug=False)
    x_d = nc.dram_tensor("x", [B_LOC, N_VAR], f32, kind="ExternalInput")
    sig_d = nc.dram_tensor("sigma2", [1, 1], f32, kind="ExternalInput")
    i16_d = nc.dram_tensor("i16_pack", list(pre["i16_pack"].shape), i16,
                           kind="ExternalInput")
    f32_d = nc.dram_tensor("f32_pack", list(pre["f32_pack"].shape), f32,
                           kind="ExternalInput")
    repl_d = nc.dram_tensor("repl", [8, 128], f32, kind="ExternalInput")
    out_d = nc.dram_tensor("out", [B_LOC, N_INFO], f32, kind="ExternalOutput")

    def i16v(ap):
        return ap.bitcast(i16)

    with tile.TileContext(nc) as tc:
        with tc.tile_pool(name="main", bufs=1) as pool, \
             tc.tile_pool(name="ps", bufs=1, space="PSUM") as psp:
            x_sb = pool.tile([B_LOC, N_VAR], f32)
            sig_sb = pool.tile([B_LOC, 1], f32)
            scale = pool.tile([B_LOC, 1], f32)
            llr = pool.tile([B_LOC, N_VAR], f32)
            i16_sb = pool.tile(list(pre["i16_pack"].shape), i16, name="i16_sb")
            f32_sb = pool.tile(list(pre["f32_pack"].shape), f32, name="f32_sb")
            repl = pool.tile([8, 128], f32)

            def i16t(nm):
                o, w = pre["i16_off"][nm]
                return i16_sb[:, o:o + w]

            def f32t(nm):
                o, w = pre["f32_off"][nm]
                return f32_sb[:, o:o + w]

            bf16 = mybir.dt.bfloat16
            Pp = pool.tile([128, CHK_PER_G], f32)
            Pfin = pool.tile([128, CHK_PER_G], f32)
            c2v = pool.tile([128, ES], f32)
            L = pool.tile([128, ES + 2], f32)
            LRUN = pool.tile([128, RP + 2], f32)
            d_t = pool.tile([128, RP], bf16)
            S = pool.tile([128, N_VAR], bf16)
            selb = pool.tile([128, 128], bf16)
            W0 = pool.tile([128, N_VAR], f32)
            W = pool.tile([128, N_VAR], f32)
            WC = pool.tile([128, RP + 2], f32)
            WCa = pool.tile([128, RP + 2], f32)
            WCb = pool.tile([128, RP + 2], f32)
            Dl = pool.tile([128, RP], f32)
            Gd = pool.tile([128, ES], f32)
            v2c = pool.tile([128, ES], f32)
            t = pool.tile([128, ES], f32)
            tcl = pool.tile([128, ES], f32)
            tp = pool.tile([128, NP], f32)
            tpf = pool.tile([128, NP], f32)
            tree = []
            w = NP // 2
            while w >= CHK_PER_G:
                tree.append(pool.tile([128, w], f32, name=f"tree{w}"))
                w //= 2
            ab2 = pool.tile([128, 2 * NP], f32)
            lab = pool.tile([128, 2 * NP], f32)
            c2vp = pool.tile([128, NP], f32)
            wfin = pool.tile([B_LOC, N_INFO], f32)
            out_sb = pool.tile([B_LOC, N_INFO], f32)
            dscr = pool.tile([128, 2], f32)     # dummy act scratch
            dscr2 = pool.tile([128, 2], f32)
            # two PSUM tiles so half-0 consumers don't wait on half-1 matmul
            Wps = [psp.tile([128, 512], f32, name=f"Wp{h}") for h in range(2)]

            # ---- loads: x/sig/repl on sync queue; packed tables on
            # vector/scalar queues so they stream in parallel.
            nc.sync.dma_start(out=x_sb[:], in_=x_d.ap())
            sig_b = bass.AP(tensor=sig_d.ap().tensor, offset=0,
                            ap=[[0, B_LOC], [1, 1]])
            nc.sync.dma_start(out=sig_sb[:], in_=sig_b)
            nc.sync.dma_start(out=repl[:], in_=repl_d.ap())
            nc.scalar.dma_start(out=i16_sb[:], in_=i16_d.ap())
            nc.sync.dma_start(out=f32_sb[:], in_=f32_d.ap())

            nc.vector.memset(dscr[:], 0.0)
            # llr = x * (-4 / sigma2)
            nc.vector.reciprocal(out=scale[:], in_=sig_sb[:])
            nc.vector.tensor_scalar_mul(out=scale[:], in0=scale[:], scalar1=-4.0)
            nc.vector.tensor_scalar_mul(out=llr[:], in0=x_sb[:], scalar1=scale[:])

            # W0 = repl @ llr  (iteration-invariant), copied out of PSUM
            for h in range(2):
                cols = slice(512 * h, 512 * (h + 1))
                nc.tensor.matmul(Wps[h][:], repl[:], llr[:, cols],
                                 start=True, stop=True)
                nc.vector.tensor_copy(out=W0[:, cols], in_=Wps[h][:])
            nc.vector.tensor_copy(out=selb[:], in_=f32t("sel"))

            # warm the Tanh table while DMAs/matmul run; write a corner of t
            # so the first real tanh (WAW on t) must follow it
            nc.scalar.activation(out=t[:, 0:2], in_=dscr[:], func=AFT.Tanh)

            def lscat(dst_fp, src_fp, idx, n_dst_fp, n_src_fp):
                nc.gpsimd.local_scatter(
                    i16v(dst_fp), i16v(src_fp), idx,
                    channels=128, num_elems=2 * n_dst_fp, num_idxs=2 * n_src_fp)

            nc.vector.memset(L[:], 0.0)
            for _rep in range(repeat):
                nc.vector.memset(c2v[:], 0.0)

                for it in range(n_iters + 1):
                    last = it == n_iters
                    if it > 0:
                        nc.vector._custom_dve(ANT_CUMSUM, out=L[:, 1:ES + 1],
                                              in0=c2v[:])
                        lscat(LRUN[:], L[:], i16t("lrun_idx"), RP + 2, ES + 2)
                        nc.vector.tensor_tensor(out=d_t[:], in0=LRUN[:, 1:RP + 1],
                                                in1=LRUN[:, 0:RP],
                                                op=mybir.AluOpType.subtract)
                        nh = 1 if last else 2
                        for h, sn in (((0, "sSA_idx"), (1, "sSB_idx"))[:nh]):
                            cols = slice(512 * h, 512 * (h + 1))
                            nc.gpsimd.local_scatter(
                                i16v(S[:, cols]), i16v(d_t[:]), i16t(sn),
                                channels=128, num_elems=512, num_idxs=RP)
                            nc.tensor.matmul(Wps[h][:], selb[:], S[:, cols],
                                             start=True, stop=True)
                    if last:
                        break
                    # W add and WC scatter split by variable halves so the
                    # DVE add of half1 overlaps the Pool scatter of half0
                    for h, (wct, wcn) in enumerate(((WCa, "wcA_idx"),
                                                    (WCb, "wcB_idx"))):
                        cols = slice(512 * h, 512 * (h + 1))
                        if it == 0:
                            wsrc = W0[:, cols]
                        else:
                            nc.vector.tensor_tensor(out=W[:, cols],
                                                    in0=W0[:, cols],
                                                    in1=Wps[h][:],
                                                    op=mybir.AluOpType.add)
                            wsrc = W[:, cols]
                        lscat(wct[:], wsrc, i16t(wcn), RP + 2, 512)
                    nc.vector.tensor_tensor(out=WC[:], in0=WCa[:], in1=WCb[:],
                                            op=mybir.AluOpType.add)
                    nc.vector.tensor_tensor(out=Dl[:], in0=WC[:, 1:RP + 1],
                                            in1=WC[:, 0:RP],
                                            op=mybir.AluOpType.subtract)
                    lscat(Gd[:], Dl[:], i16t("gd_idx"), ES, RP)
                    nc.vector._custom_dve(ANT_SCANSUB, out=v2c[:], in0=Gd[:],
                                          in1=c2v[:])
                    nc.scalar.activation(out=t[:], in_=v2c[:],
                                         func=AFT.Tanh, scale=0.5)
                    # warm the Ln table during the slot-domain chain: reads t
                    # (after real tanh) and writes a corner of lab (WAW forces
                    # the real Ln after it)
                    nc.scalar.activation(out=lab[:, 0:2], in_=t[:, 0:2],
                                         func=AFT.Ln)
                    nc.vector._custom_dve(ANT_SGNCLIP, out=tcl[:], in0=t[:],
                                          s0=EPS, s1=CLIP)
                    lscat(tp[:], tcl[:], i16t("tp_idx"), NP, ES)
                    nc.vector.tensor_tensor(out=tpf[:], in0=tp[:], in1=f32t("padc"),
                                            op=mybir.AluOpType.add)
                    src, dd = tpf, D_PAD
                    for rbuf in tree:
                        s3 = src[:].rearrange("p (c d) -> p c d", d=dd)
                        nc.vector.tensor_tensor(out=rbuf[:], in0=s3[:, :, 0::2],
                                                in1=s3[:, :, 1::2],
                                                op=mybir.AluOpType.mult)
                        src, dd = rbuf, dd // 2
                    P = src
                    if pre["has_split"]:
                        lscat(Pp[:], P[:], i16t("pp_idx"), CHK_PER_G, CHK_PER_G)
                        nc.vector.tensor_tensor(out=Pp[:], in0=Pp[:],
                                                in1=f32t("mask1"),
                                                op=mybir.AluOpType.add)
                        nc.vector.tensor_tensor(out=Pfin[:], in0=P[:], in1=Pp[:],
                                                op=mybir.AluOpType.mult)
                        P = Pfin
                    # sq -> Ln -> subclip split by slot halves: ACT Ln of
                    # half0 overlaps DVE sq of half1. Layout per half hh:
                    # ab2/lab[:, 2*hh*NPH : 2*(hh+1)*NPH] = [a2 | b2]
                    NPH = NP // 2
                    CPH = CHK_PER_G // 2
                    for hh in range(2):
                        sl = slice(NPH * hh, NPH * (hh + 1))
                        o = 2 * NPH * hh
                        Ph = P[:, CPH * hh:CPH * (hh + 1)]
                        Pb = bass.AP(tensor=Ph.tensor, offset=Ph.offset,
                                     ap=[Ph.ap[0], [1, CPH], [0, D_PAD]])
                        nc.vector._custom_dve(ANT_SQADD, out=ab2[:, o:o + NPH],
                                              in0=tpf[:, sl], in1=Pb)
                        nc.vector._custom_dve(ANT_SQSUB,
                                              out=ab2[:, o + NPH:o + 2 * NPH],
                                              in0=tpf[:, sl], in1=Pb)
                        nc.scalar.activation(out=lab[:, o:o + 2 * NPH],
                                             in_=ab2[:, o:o + 2 * NPH],
                                             func=AFT.Ln)
                    # warm the next table (Tanh, or Sigmoid on the last iter):
                    # reads lab (after real Ln), writes a corner of the next
                    # real activation's output tile
                    if it == n_iters - 1:
                        nc.scalar.activation(out=out_sb[:, 0:2],
                                             in_=lab[0:B_LOC, 0:2],
                                             func=AFT.Sigmoid)
                    else:
                        nc.scalar.activation(out=t[:, 0:2], in_=lab[:, 0:2],
                                             func=AFT.Tanh)
                    for hh in range(2):
                        o = 2 * NPH * hh
                        nc.vector._custom_dve(ANT_SUBCLIPSCALE,
                                              out=c2vp[:, NPH * hh:NPH * (hh + 1)],
                                              in0=lab[:, o:o + NPH],
                                              in1=lab[:, o + NPH:o + 2 * NPH],
                                              s0=-C2V_BOUND, s1=C2V_BOUND,
                                              imm2=0.5)
                    lscat(c2v[:], c2vp[:], i16t("bk_idx"), ES, NP)

                nc.vector.tensor_tensor(out=wfin[:], in0=W0[0:B_LOC, 0:N_INFO],
                                        in1=Wps[0][0:B_LOC, :],
                                        op=mybir.AluOpType.add)
                nc.scalar.activation(out=out_sb[:], in_=wfin[:],
                                     func=AFT.Sigmoid, scale=-1.0)
            nc.sync.dma_start(out=out_d.ap(), in_=out_sb[:])
    nc.compile()
    return nc


def make_in_maps(pre, x, sigma2):
    sigma2 = np.asarray(sigma2, np.float32).reshape(1, 1)
    in_maps = []
    for c in range(N_CORES):
        m = {"x": np.asarray(x[c * B_LOC:(c + 1) * B_LOC], np.float32),
             "sigma2": sigma2,
             "i16_pack": pre["i16_pack"],
             "f32_pack": pre["f32_pack"],
             "repl": pre["repl"]}
        in_maps.append(m)
    return in_maps


# ------------------------------------------------------------------ reference
def ref_numpy(inputs, H, sigma2, n_iter=N_ITER):
    llr = -4.0 * inputs / np.asarray(sigma2).reshape(-1)
    B = inputs.shape[0]
    mask = H[None]
    c2v = np.zeros((B, N_CHK, N_VAR), np.float32)
    for _ in range(n_iter):
        v2c = (llr[:, None, :] + c2v.sum(1, keepdims=True) - c2v) * mask
        tt = np.tanh(0.5 * v2c)
        tt = np.where(mask > 0, tt, 1.0)
        sgn = np.where(tt >= 0, 1.0, -1.0)
        tt = sgn * np.clip(np.abs(tt), EPS, CLIP)
        ext = np.prod(tt, axis=2, keepdims=True) / tt
        c2v = 2.0 * np.arctanh(np.clip(ext, -CLIP, CLIP)) * mask
    total = llr + c2v.sum(1)
    return (1.0 / (1.0 + np.exp(total[:, :N_INFO]))).astype(np.float32)


# ------------------------------------------------------------------- pipeline
def run(inputs, H, sigma2, n_iters=N_ITER, repeat=1, nc=None, pre=None):
    inputs = np.asarray(inputs, np.float32)
    H = np.asarray(H, np.float32)
    if pre is None:
        pre = prep(H)
    if nc is None:
        nc = build_nc(pre, n_iters=n_iters, repeat=repeat)
    in_maps = make_in_maps(pre, inputs, sigma2)
    res = bass_utils.run_bass_kernel_spmd(nc, in_maps, core_ids=list(range(N_CORES)))
    out = np.concatenate([res.results[c]["out"] for c in range(N_CORES)], axis=0)
    return out


# ------------------------------------------------------------------ entry
_CACHE = {}


def kernel(inputs, H, sigma2):
    """Full-input entry point: inputs [64,1024] f32, H [512,1024] f32,
    sigma2 [1] f32 -> [64, 512] f32 (matches reference.reference)."""
    inputs = np.ascontiguousarray(np.asarray(inputs, np.float32))
    H = np.asarray(H, np.float32)
    pre = prep(H)
    key = (pre["ES"], pre["RP"], pre["NP"], pre["has_split"])
    nc = _CACHE.get(key)
    if nc is None:
        nc = build_nc(pre)
        _CACHE[key] = nc
    out = run(inputs, H, sigma2, nc=nc, pre=pre)
    return np.ascontiguousarray(out.astype(np.float32))


# revision 33
# speedup vs baseline: 319.2645x; 3.1977x over previous
"""Belief-propagation decoder kernel for TRN2 (8 NeuronCores, data-parallel batch).

v3: 16 check-groups x 8 batch lanes (v2 duplicated each batch twice across
16 lanes/group — pure waste on GPSIMD/DVE). All stream widths (ES/RP/NP)
halve. repl@llr hoisted out of the loop (W0). Activation table loads hidden
behind dummy activations on the idle ACT engine. Index tables packed into a
single DMA issued on a second queue.

Layout: 128 partitions = 16 check-groups x 8 lanes (batch b = lane). Each
group owns ~32 checks (~192 edges). Per group the edge stream is v-sorted;
"runs" are maximal same-v segments, r = 0..R-1, run r covers stream
[a_r, a_{r+1}), variable v(r) strictly increasing.

Per iteration:
  L = cumsum(c2v)                      [DVE scan]
  LRUN[r] = L[a_r]                     [local_scatter]
  d[r] = LRUN[r+1]-LRUN[r]             [DVE] = per-group run sums
  S[v(r)] = d[r] (0 elsewhere)         [2x local_scatter halves]
  Wp = sel@S                           [PE matmul, PSUM] = sum_g S_g
  W = W0 + Wp                          [DVE]  (W0 = repl@llr, hoisted)
  WC[r+1] = W[v(r)]                    [local_scatter]
  Dl[r] = WC[r+1]-WC[r]                [DVE]
  Gd[a_r] = Dl[r] (0 elsewhere)        [local_scatter]
  v2c = cumsum(Gd) - c2v               [DVE custom scan-sub] = W(v(e)) - c2v
  t = tanh(0.5*v2c)                    [ACT]
  tc = sgn*clip(|t|)                   [DVE custom]
  tp[slot(e)] = tc[e]                  [local_scatter]
  tpf = tp + padconst                  [DVE]  (padconst: K_c / 1.0 at pads)
  P = product-tree over D slots        [DVE x log2(D)]
  a2 = (tpf+P)^2, b2 = (tpf-P)^2       [DVE custom, P broadcast]
  La = Ln(a2), Lb = Ln(b2)             [ACT]
  c2v_p = clip(0.5*(La-Lb))            [DVE custom] = 2*atanh(clip(P/t))
  c2v[e] = c2v_p[slot(e)]              [local_scatter]
"""
import sys, os
sys.path.insert(0, "/opt/trn_rl_repo")
import numpy as np

import concourse.bass as bass
import concourse.bacc as bacc
import concourse.tile as tile
from concourse import mybir
from concourse import bass_utils

# ----------------------------------------------------------------- constants
N_VAR, N_CHK, N_INFO, N_ITER, BATCH = 1024, 512, 512, 5, 64
EPS = 1e-7
CLIP = 1.0 - 1e-6
C2V_BOUND = float(2.0 * np.arctanh(np.float32(CLIP)))
N_CORES = 8
N_GROUPS = 16
CHK_PER_G = 34          # check SLOTS per group; big checks (deg>15) use 2
D_PAD = 16
B_LOC = 8

# ------------------------------------------------------- custom DVE ops
from concourse.dve_spec import (
    Spec, Src0, Src1, C0, C1, C2, Zero, One, scan, AluOp,
    select, maxx, minn, sq, lower, _has_src1 as has_src1,
)
import concourse.dve_ops as dve_ops
from concourse.dve_ops import DveOp, OPS
from concourse.dve_uop import DveOpSpec


def _register(name, spec, subdim=False):
    if name in dve_ops._SUB_OPCODE_FOR_NAME:
        for op in OPS:
            if op.name == name:
                return op
        raise RuntimeError(name)
    shas = {}
    for ver in ("v3", "v4"):
        try:
            tmp = DveOpSpec(name=name, uops=lower(spec, ver=ver),
                            rd1_en=has_src1(spec))
            shas[ver] = tmp.sha(ver)
        except Exception:
            pass
    op = DveOp(name, spec, subdim=subdim, uops_sha=shas)
    OPS.append(op)
    dve_ops.CUSTOM_DVE_SPECS[name] = spec
    dve_ops._SUB_OPCODE_FOR_NAME[name] = dve_ops._CUSTOM_DVE_ROW_BASE + len(OPS) - 1
    assert dve_ops._SUB_OPCODE_FOR_NAME[name] < 0x20
    return op


ANT_CUMSUM = _register("ANT_BP_CUMSUM", Spec(
    body=scan(AluOp.ADD, Src0),
    reference=lambda in0, in1, s0, s1, imm2: np.cumsum(in0, axis=-1),
))
# out = cumsum(in0) - in1
ANT_SCANSUB = _register("ANT_BP_SCANSUB", Spec(
    body=scan(AluOp.ADD, Src0) - Src1,
    reference=lambda in0, in1, s0, s1, imm2: np.cumsum(in0, axis=-1) - in1,
))
# sgn(x)*clip(|x|, s0, s1), sgn(-0.0)=+1 (matches jnp.where(t >= 0, 1, -1))
ANT_SGNCLIP = _register("ANT_BP_SGNCLIP", Spec(
    body=select(Src0 < Zero, Zero - One, One)
         * minn(maxx(maxx(Src0, Zero - Src0), C0), C1),
    reference=lambda in0, in1, s0, s1, imm2:
        np.where(in0 < 0, -1.0, 1.0).astype(np.float32)
        * np.clip(np.abs(in0), s0, s1),
))
ANT_SQADD = _register("ANT_BP_SQADD", Spec(
    body=sq(Src0 + Src1),
    reference=lambda in0, in1, s0, s1, imm2: (in0 + in1.reshape(in0.shape)) ** 2,
))
ANT_SQSUB = _register("ANT_BP_SQSUB", Spec(
    body=sq(Src0 - Src1),
    reference=lambda in0, in1, s0, s1, imm2: (in0 - in1.reshape(in0.shape)) ** 2,
))
# clip((in0-in1)*imm2, s0, s1)
ANT_SUBCLIPSCALE = _register("ANT_BP_SUBCLIPSCALE", Spec(
    body=minn(maxx((Src0 - Src1) * C2, C0), C1),
    reference=lambda in0, in1, s0, s1, imm2: np.clip((in0 - in1) * imm2, s0, s1),
))


# ------------------------------------------------------------ host-side prep
def prep(H: np.ndarray) -> dict:
    H = np.asarray(H)
    assert H.shape == (N_CHK, N_VAR)
    deg = (H > 0).sum(1).astype(int)
    assert deg.max() <= 30
    NP = CHK_PER_G * D_PAD
    slots_of = {c: (2 if deg[c] > 15 else 1) for c in range(N_CHK)}
    order = np.argsort(-deg, kind="stable")
    ge = [0] * N_GROUPS
    gcnt = [0] * N_GROUPS
    grp = np.zeros(N_CHK, int)
    for c in order:
        g = min(range(N_GROUPS),
                key=lambda g: ge[g]
                if gcnt[g] + slots_of[c] <= CHK_PER_G else 1 << 30)
        grp[c] = g
        ge[g] += deg[c]
        gcnt[g] += slots_of[c]
    ES = -(-max(max(ge), 1) // 2) * 2          # stream length (even)

    # per-group metadata
    g_edges = []    # (checks, sorted (v, c) list)
    g_runs = []     # ([(a_r, v_r)...], Eg)
    for g in range(N_GROUPS):
        checks = np.where(grp == g)[0]
        es = []
        for c in checks:
            for v in np.where(H[c] > 0)[0]:
                es.append((int(v), int(c)))
        es.sort()
        g_edges.append((checks, es))
        runs = []
        for e, (v, c) in enumerate(es):
            if not runs or runs[-1][1] != v:
                runs.append((e, v))
        g_runs.append((runs, len(es)))
    R_max = max(len(r) for r, _ in g_runs)
    RP = -(-R_max // 2) * 2                     # run slots, even

    def pairs(n):
        return np.full((128, 2 * n), -1, np.int16)

    lrun_idx = pairs(ES + 2)        # source L[0..ES+1] -> LRUN[r] at a_r
    sSA_idx = np.full((128, RP), -1, np.int16)  # bf16: d[r] -> S[v(r)], v<512
    sSB_idx = np.full((128, RP), -1, np.int16)  # bf16: d[r] -> S[v(r)-512]
    wcA_idx = np.full((128, 512), -1, np.int16)  # bf16 W[0:512][v] -> WCa[r+1]
    wcB_idx = np.full((128, 512), -1, np.int16)  # bf16 W[512:][v-512] -> WCb[r+1]
    gd_idx = pairs(RP)              # source Dl[r] -> Gd[a_r]
    tp_idx = pairs(ES)              # source tc[e] -> tp slot
    bk_idx = pairs(NP)              # source c2v_p[slot] -> c2v[e]
    padc = np.zeros((128, NP), np.float32)
    partner = np.full((128, CHK_PER_G), -1, np.int64)
    mask1 = np.ones((128, CHK_PER_G), np.float32)

    for g in range(N_GROUPS):
        checks, es = g_edges[g]
        runs, Eg = g_runs[g]
        rows = slice(B_LOC * g, B_LOC * g + B_LOC)

        def put(arr, src_fp, dst_fp):
            arr[rows, 2 * src_fp] = 2 * dst_fp
            arr[rows, 2 * src_fp + 1] = 2 * dst_fp + 1

        for r, (a_r, v_r) in enumerate(runs):
            put(lrun_idx, a_r, r)
            if v_r < 512:
                sSA_idx[rows, r] = v_r
            else:
                sSB_idx[rows, r] = v_r - 512
            if v_r < 512:
                wcA_idx[rows, v_r] = r + 1
            else:
                wcB_idx[rows, v_r - 512] = r + 1
            put(gd_idx, r, a_r)
        put(lrun_idx, Eg, len(runs))            # closing boundary

        # assign slot positions: big checks take slot pairs (i, i+1)
        cpos = {}
        nxt_slot = 0
        for c in checks:
            cpos[c] = nxt_slot
            if slots_of[c] == 2:
                partner[rows, nxt_slot] = nxt_slot + 1
                partner[rows, nxt_slot + 1] = nxt_slot
                mask1[rows, nxt_slot] = 0.0
                mask1[rows, nxt_slot + 1] = 0.0
            nxt_slot += slots_of[c]
        assert nxt_slot <= CHK_PER_G
        dslot = {c: 0 for c in checks}
        for e, (v, c) in enumerate(es):
            s = cpos[c] * D_PAD + dslot[c]
            dslot[c] += 1
            put(tp_idx, e, s)
            put(bk_idx, s, e)
        for c in checks:
            dd = dslot[c]            # = deg(c), spans 1 or 2 slots
            base = cpos[c] * D_PAD
            k = np.float32(np.float64(CLIP) ** (N_VAR - dd))
            padc[rows, base + dd] = k
            for j in range(dd + 1, slots_of[c] * D_PAD):
                padc[rows, base + j] = 1.0

    # selector matrices: partition p = 8g + b, batch lane b = p % 8
    sel = np.zeros((128, 128), np.float32)
    for k in range(128):
        for m in range(128):
            if m % B_LOC == k % B_LOC:
                sel[k, m] = 1.0
    repl = np.zeros((8, 128), np.float32)
    for b in range(8):
        for m in range(128):
            if m % B_LOC == b:
                repl[b, m] = 1.0

    has_split = bool((mask1 == 0.0).any())
    pp_idx = np.full((128, 2 * CHK_PER_G), -1, np.int16)
    for p in range(128):
        for sl in range(CHK_PER_G):
            if partner[p, sl] >= 0:
                pp_idx[p, 2 * sl] = 2 * partner[p, sl]
                pp_idx[p, 2 * sl + 1] = 2 * partner[p, sl] + 1

    # ---- packed tables: one i16 blob + one f32 blob (single DMAs)
    i16_parts = [("lrun_idx", lrun_idx), ("sSA_idx", sSA_idx),
                 ("sSB_idx", sSB_idx),
                 ("wcA_idx", wcA_idx), ("wcB_idx", wcB_idx),
                 ("gd_idx", gd_idx), ("tp_idx", tp_idx),
                 ("bk_idx", bk_idx), ("pp_idx", pp_idx)]
    i16_off = {}
    off = 0
    for nm, arr in i16_parts:
        i16_off[nm] = (off, arr.shape[1])
        off += arr.shape[1]
    i16_pack = np.concatenate([a for _, a in i16_parts], axis=1)
    f32_parts = [("padc", padc), ("mask1", mask1), ("sel", sel)]
    f32_off = {}
    off = 0
    for nm, arr in f32_parts:
        f32_off[nm] = (off, arr.shape[1])
        off += arr.shape[1]
    f32_pack = np.concatenate([a for _, a in f32_parts], axis=1)

    return dict(ES=ES, RP=RP, NP=NP, D_PAD=D_PAD, has_split=has_split,
                i16_pack=i16_pack, i16_off=i16_off,
                f32_pack=f32_pack, f32_off=f32_off,
                repl=repl,
                _dbg=dict(g_edges=g_edges, g_runs=g_runs))


# ------------------------------------------------------------- device program
def build_nc(pre: dict, n_iters: int = N_ITER, repeat: int = 1):
    ES, RP, NP = pre["ES"], pre["RP"], pre["NP"]
    f32 = mybir.dt.float32
    i16 = mybir.dt.int16
    AFT = mybir.ActivationFunctionType

    nc = bacc.Bacc("TRN2", target_bir_lowering=False, debug=False)
    x_d = nc.dram_tensor("x", [B_LOC, N_VAR], f32, kind="ExternalInput")
    sig_d = nc.dram_tensor("sigma2", [1, 1], f32, kind="ExternalInput")
    i16_d = nc.dram_tensor("i16_pack", list(pre["i16_pack"].shape), i16,
                           kind="ExternalInput")
    f32_d = nc.dram_tensor("f32_pack", list(pre["f32_pack"].shape), f32,
                           kind="ExternalInput")
    repl_d = nc.dram_tensor("repl", [8, 128], f32, kind="ExternalInput")
    out_d = nc.dram_tensor("out", [B_LOC, N_INFO], f32, kind="ExternalOutput")

    def i16v(ap):
        return ap.bitcast(i16)

    with tile.TileContext(nc) as tc:
        with tc.tile_pool(name="main", bufs=1) as pool, \
             tc.tile_pool(name="ps", bufs=1, space="PSUM") as psp:
            x_sb = pool.tile([B_LOC, N_VAR], f32)
            sig_sb = pool.tile([B_LOC, 1], f32)
            scale = pool.tile([B_LOC, 1], f32)
            llr = pool.tile([B_LOC, N_VAR], f32)
            i16_sb = pool.tile(list(pre["i16_pack"].shape), i16, name="i16_sb")
            f32_sb = pool.tile(list(pre["f32_pack"].shape), f32, name="f32_sb")
            repl = pool.tile([8, 128], f32)

            def i16t(nm):
                o, w = pre["i16_off"][nm]
                return i16_sb[:, o:o + w]

            def f32t(nm):
                o, w = pre["f32_off"][nm]
                return f32_sb[:, o:o + w]

            bf16 = mybir.dt.bfloat16
            Pp = pool.tile([128, CHK_PER_G], f32)
            Pfin = pool.tile([128, CHK_PER_G], f32)
            c2v = pool.tile([128, ES], f32)
            L = pool.tile([128, ES + 2], f32)
            LRUN = pool.tile([128, RP + 2], f32)
            d_t = pool.tile([128, RP], bf16)
            S = pool.tile([128, N_VAR], bf16)
            selb = pool.tile([128, 128], bf16)
            W0 = pool.tile([128, N_VAR], f32)
            W0b = pool.tile([128, N_VAR], bf16)
            W = pool.tile([128, N_VAR], bf16)
            WC = pool.tile([128, RP + 2], f32)
            WCa = pool.tile([128, RP + 2], bf16)
            WCb = pool.tile([128, RP + 2], bf16)
            Dl = pool.tile([128, RP], f32)
            Gd = pool.tile([128, ES], f32)
            v2c = pool.tile([128, ES], f32)
            t = pool.tile([128, ES], f32)
            tcl = pool.tile([128, ES], f32)
            tp = pool.tile([128, NP], f32)
            tpf = pool.tile([128, NP], f32)
            tree = []
            w = NP // 2
            while w >= CHK_PER_G:
                tree.append(pool.tile([128, w], f32, name=f"tree{w}"))
                w //= 2
            ab2 = pool.tile([128, 2 * NP], f32)
            lab = pool.tile([128, 2 * NP], f32)
            c2vp = pool.tile([128, NP], f32)
            wfin = pool.tile([B_LOC, N_INFO], f32)
            out_sb = pool.tile([B_LOC, N_INFO], f32)
            dscr = pool.tile([128, 2], f32)     # dummy act scratch
            dscr2 = pool.tile([128, 2], f32)
            # two PSUM tiles so half-0 consumers don't wait on half-1 matmul
            Wps = [psp.tile([128, 512], f32, name=f"Wp{h}") for h in range(2)]

            # ---- loads: x/sig/repl on sync queue; packed tables on
            # vector/scalar queues so they stream in parallel.
            nc.sync.dma_start(out=x_sb[:], in_=x_d.ap())
            sig_b = bass.AP(tensor=sig_d.ap().tensor, offset=0,
                            ap=[[0, B_LOC], [1, 1]])
            nc.sync.dma_start(out=sig_sb[:], in_=sig_b)
            nc.sync.dma_start(out=repl[:], in_=repl_d.ap())
            nc.scalar.dma_start(out=i16_sb[:], in_=i16_d.ap())
            nc.sync.dma_start(out=f32_sb[:], in_=f32_d.ap())

            nc.vector.memset(dscr[:], 0.0)
            # llr = x * (-4 / sigma2)
            nc.vector.reciprocal(out=scale[:], in_=sig_sb[:])
            nc.vector.tensor_scalar_mul(out=scale[:], in0=scale[:], scalar1=-4.0)
            nc.vector.tensor_scalar_mul(out=llr[:], in0=x_sb[:], scalar1=scale[:])

            # W0 = repl @ llr  (iteration-invariant), copied out of PSUM
            for h in range(2):
                cols = slice(512 * h, 512 * (h + 1))
                nc.tensor.matmul(Wps[h][:], repl[:], llr[:, cols],
                                 start=True, stop=True)
                nc.vector.tensor_copy(out=W0[:, cols], in_=Wps[h][:])
                nc.vector.tensor_copy(out=W0b[:, cols], in_=Wps[h][:])
            nc.vector.tensor_copy(out=selb[:], in_=f32t("sel"))

            # warm the Tanh table while DMAs/matmul run; write a corner of t
            # so the first real tanh (WAW on t) must follow it
            nc.scalar.activation(out=t[:, 0:2], in_=dscr[:], func=AFT.Tanh)

            def lscat(dst_fp, src_fp, idx, n_dst_fp, n_src_fp):
                nc.gpsimd.local_scatter(
                    i16v(dst_fp), i16v(src_fp), idx,
                    channels=128, num_elems=2 * n_dst_fp, num_idxs=2 * n_src_fp)

            nc.vector.memset(L[:], 0.0)
            for _rep in range(repeat):
                nc.vector.memset(c2v[:], 0.0)

                for it in range(n_iters + 1):
                    last = it == n_iters
                    if it > 0:
                        nc.vector._custom_dve(ANT_CUMSUM, out=L[:, 1:ES + 1],
                                              in0=c2v[:])
                        lscat(LRUN[:], L[:], i16t("lrun_idx"), RP + 2, ES + 2)
                        nc.vector.tensor_tensor(out=d_t[:], in0=LRUN[:, 1:RP + 1],
                                                in1=LRUN[:, 0:RP],
                                                op=mybir.AluOpType.subtract)
                        nh = 1 if last else 2
                        for h, sn in (((0, "sSA_idx"), (1, "sSB_idx"))[:nh]):
                            cols = slice(512 * h, 512 * (h + 1))
                            nc.gpsimd.local_scatter(
                                i16v(S[:, cols]), i16v(d_t[:]), i16t(sn),
                                channels=128, num_elems=512, num_idxs=RP)
                            nc.tensor.matmul(Wps[h][:], selb[:], S[:, cols],
                                             start=True, stop=True)
                    if last:
                        break
                    # W add and WC scatter split by variable halves so the
                    # DVE add of half1 overlaps the Pool scatter of half0
                    for h, (wct, wcn) in enumerate(((WCa, "wcA_idx"),
                                                    (WCb, "wcB_idx"))):
                        cols = slice(512 * h, 512 * (h + 1))
                        if it == 0:
                            wsrc = W0b[:, cols]
                        else:
                            nc.vector.tensor_tensor(out=W[:, cols],
                                                    in0=W0[:, cols],
                                                    in1=Wps[h][:],
                                                    op=mybir.AluOpType.add)
                            wsrc = W[:, cols]
                        nc.gpsimd.local_scatter(
                            i16v(wct[:]), i16v(wsrc), i16t(wcn),
                            channels=128, num_elems=RP + 2, num_idxs=512)
                    nc.vector.tensor_tensor(out=WC[:], in0=WCa[:], in1=WCb[:],
                                            op=mybir.AluOpType.add)
                    nc.vector.tensor_tensor(out=Dl[:], in0=WC[:, 1:RP + 1],
                                            in1=WC[:, 0:RP],
                                            op=mybir.AluOpType.subtract)
                    lscat(Gd[:], Dl[:], i16t("gd_idx"), ES, RP)
                    nc.vector._custom_dve(ANT_SCANSUB, out=v2c[:], in0=Gd[:],
                                          in1=c2v[:])
                    nc.scalar.activation(out=t[:], in_=v2c[:],
                                         func=AFT.Tanh, scale=0.5)
                    # warm the Ln table during the slot-domain chain: reads t
                    # (after real tanh) and writes a corner of lab (WAW forces
                    # the real Ln after it)
                    nc.scalar.activation(out=lab[:, 0:2], in_=t[:, 0:2],
                                         func=AFT.Ln)
                    nc.vector._custom_dve(ANT_SGNCLIP, out=tcl[:], in0=t[:],
                                          s0=EPS, s1=CLIP)
                    lscat(tp[:], tcl[:], i16t("tp_idx"), NP, ES)
                    nc.vector.tensor_tensor(out=tpf[:], in0=tp[:], in1=f32t("padc"),
                                            op=mybir.AluOpType.add)
                    src, dd = tpf, D_PAD
                    for rbuf in tree:
                        s3 = src[:].rearrange("p (c d) -> p c d", d=dd)
                        nc.vector.tensor_tensor(out=rbuf[:], in0=s3[:, :, 0::2],
                                                in1=s3[:, :, 1::2],
                                                op=mybir.AluOpType.mult)
                        src, dd = rbuf, dd // 2
                    P = src
                    if pre["has_split"]:
                        lscat(Pp[:], P[:], i16t("pp_idx"), CHK_PER_G, CHK_PER_G)
                        nc.vector.tensor_tensor(out=Pp[:], in0=Pp[:],
                                                in1=f32t("mask1"),
                                                op=mybir.AluOpType.add)
                        nc.vector.tensor_tensor(out=Pfin[:], in0=P[:], in1=Pp[:],
                                                op=mybir.AluOpType.mult)
                        P = Pfin
                    # sq -> Ln -> subclip split by slot halves: ACT Ln of
                    # half0 overlaps DVE sq of half1. Layout per half hh:
                    # ab2/lab[:, 2*hh*NPH : 2*(hh+1)*NPH] = [a2 | b2]
                    NPH = NP // 2
                    CPH = CHK_PER_G // 2
                    for hh in range(2):
                        sl = slice(NPH * hh, NPH * (hh + 1))
                        o = 2 * NPH * hh
                        Ph = P[:, CPH * hh:CPH * (hh + 1)]
                        Pb = bass.AP(tensor=Ph.tensor, offset=Ph.offset,
                                     ap=[Ph.ap[0], [1, CPH], [0, D_PAD]])
                        nc.vector._custom_dve(ANT_SQADD, out=ab2[:, o:o + NPH],
                                              in0=tpf[:, sl], in1=Pb)
                        nc.vector._custom_dve(ANT_SQSUB,
                                              out=ab2[:, o + NPH:o + 2 * NPH],
                                              in0=tpf[:, sl], in1=Pb)
                        nc.scalar.activation(out=lab[:, o:o + 2 * NPH],
                                             in_=ab2[:, o:o + 2 * NPH],
                                             func=AFT.Ln)
                    # warm the next table (Tanh, or Sigmoid on the last iter):
                    # reads lab (after real Ln), writes a corner of the next
                    # real activation's output tile
                    if it == n_iters - 1:
                        nc.scalar.activation(out=out_sb[:, 0:2],
                                             in_=lab[0:B_LOC, 0:2],
                                             func=AFT.Sigmoid)
                    else:
                        nc.scalar.activation(out=t[:, 0:2], in_=lab[:, 0:2],
                                             func=AFT.Tanh)
                    for hh in range(2):
                        o = 2 * NPH * hh
                        nc.vector._custom_dve(ANT_SUBCLIPSCALE,
                                              out=c2vp[:, NPH * hh:NPH * (hh + 1)],
                                              in0=lab[:, o:o + NPH],
                                              in1=lab[:, o + NPH:o + 2 * NPH],
                                              s0=-C2V_BOUND, s1=C2V_BOUND,
                                              imm2=0.5)
                    lscat(c2v[:], c2vp[:], i16t("bk_idx"), ES, NP)

                nc.vector.tensor_tensor(out=wfin[:], in0=W0[0:B_LOC, 0:N_INFO],
                                        in1=Wps[0][0:B_LOC, :],
                                        op=mybir.AluOpType.add)
                nc.scalar.activation(out=out_sb[:], in_=wfin[:],
                                     func=AFT.Sigmoid, scale=-1.0)
            nc.sync.dma_start(out=out_d.ap(), in_=out_sb[:])
    nc.compile()
    return nc


def make_in_maps(pre, x, sigma2):
    sigma2 = np.asarray(sigma2, np.float32).reshape(1, 1)
    in_maps = []
    for c in range(N_CORES):
        m = {"x": np.asarray(x[c * B_LOC:(c + 1) * B_LOC], np.float32),
             "sigma2": sigma2,
             "i16_pack": pre["i16_pack"],
             "f32_pack": pre["f32_pack"],
             "repl": pre["repl"]}
        in_maps.append(m)
    return in_maps


# ------------------------------------------------------------------ reference
def ref_numpy(inputs, H, sigma2, n_iter=N_ITER):
    llr = -4.0 * inputs / np.asarray(sigma2).reshape(-1)
    B = inputs.shape[0]
    mask = H[None]
    c2v = np.zeros((B, N_CHK, N_VAR), np.float32)
    for _ in range(n_iter):
        v2c = (llr[:, None, :] + c2v.sum(1, keepdims=True) - c2v) * mask
        tt = np.tanh(0.5 * v2c)
        tt = np.where(mask > 0, tt, 1.0)
        sgn = np.where(tt >= 0, 1.0, -1.0)
        tt = sgn * np.clip(np.abs(tt), EPS, CLIP)
        ext = np.prod(tt, axis=2, keepdims=True) / tt
        c2v = 2.0 * np.arctanh(np.clip(ext, -CLIP, CLIP)) * mask
    total = llr + c2v.sum(1)
    return (1.0 / (1.0 + np.exp(total[:, :N_INFO]))).astype(np.float32)


# ------------------------------------------------------------------- pipeline
def run(inputs, H, sigma2, n_iters=N_ITER, repeat=1, nc=None, pre=None):
    inputs = np.asarray(inputs, np.float32)
    H = np.asarray(H, np.float32)
    if pre is None:
        pre = prep(H)
    if nc is None:
        nc = build_nc(pre, n_iters=n_iters, repeat=repeat)
    in_maps = make_in_maps(pre, inputs, sigma2)
    res = bass_utils.run_bass_kernel_spmd(nc, in_maps, core_ids=list(range(N_CORES)))
    out = np.concatenate([res.results[c]["out"] for c in range(N_CORES)], axis=0)
    return out


# ------------------------------------------------------------------ entry
_CACHE = {}


def kernel(inputs, H, sigma2):
    """Full-input entry point: inputs [64,1024] f32, H [512,1024] f32,
    sigma2 [1] f32 -> [64, 512] f32 (matches reference.reference)."""
    inputs = np.ascontiguousarray(np.asarray(inputs, np.float32))
    H = np.asarray(H, np.float32)
    pre = prep(H)
    key = (pre["ES"], pre["RP"], pre["NP"], pre["has_split"])
    nc = _CACHE.get(key)
    if nc is None:
        nc = build_nc(pre)
        _CACHE[key] = nc
    out = run(inputs, H, sigma2, nc=nc, pre=pre)
    return np.ascontiguousarray(out.astype(np.float32))
